# revision 12
# baseline (speedup 1.0000x reference)
"""GAT-style masked self-attention (B=4, N=4096, D=128) on 8 trn2 NeuronCores.

reference:
    scores = X @ X^T / sqrt(D)            [B, N, N]
    masked = where(adj > 0, scores, -1e12)
    attn   = softmax(masked, axis=2)
    out    = attn @ X                     [B, N, D]

Sharding: 8 cores <- (batch b, row-half h); each core handles 2048 rows
of one batch element against all 4096 keys. No collectives.

Device algorithm (per core), orientation "S^T" (keys on partitions).
X is host-prescaled by sqrt(ALPHA) so the score matmul's PSUM value is
directly the fast-exp fixed-point argument psS = raw_dot*ALPHA.

The mask is applied via THREE balanced paths (chosen to put every
engine at the PE roofline; GPSIMD tensor ops are avoided entirely --
they starve DVE of SBUF bandwidth, ~4x slowdown on concurrent ops):

  k-tiles 26..31 (NK_F=6), DVE fast-exp with the mask baked in:
      u16 = convert(psS + madd)   [one tensor_tensor add]
    madd (host int16) = unmasked ? (C + 1024*bit(row)) : -32768.
    Unmasked: Schraudolph fast exp -- the u16 bit pattern read as fp16
    is 2^bit(row) * exp(raw_dot*SCALE - 8); masked: the sum is negative
    and the u16 convert clamps to 0. The host clamps diagonal entries
    so the sum stays below the fp16 inf bit region. The result feeds
    the AV matmul directly (bitcast fp16), no separate mask multiply.

  k-tiles 0..12 (NK_U=13): mask bits host-packed 8 rows per uint16 word
    (bit i of word j in a 512-row chunk = row i*64+j); DVE unpacks with
    one tensor_scalar bitwise_and per bit (4x DVE mode) into
    m_u in {0, 2^bit(row)}.

  k-tiles 13..25: m_u values sent fully-formed from the host (DMA is
    far under its roofline after bit-packing, DVE is not).

  ACT evicts k-tiles 0..25 with exp fused (output bfloat16; the wide
  exponent absorbs the 2^bit row constants), then DVE multiplies
  ptm = p * m_u (mixed bf16 x u16 -> bf16, 2x mode). The per-row
  constant 2^bit(row) is identical across all keys of a row, so it
  cancels in the softmax ratio.

  AV matmul with the denominator fused via an appended ones-column:
      psO[rc] (+)= w_k.T @ [X_k | 1] over k in (26..31, 0..25)
      w_k = pef (fp16 bitcast) for fexp tiles, ptm (bf16) otherwise
      out = psO[:, :128] * (1 / psO[:, 128]) -- recip on DVE, the
      normalize multiply on ACT (Copy activation with scale AP)

  Row blocks are software-pipelined: block i runs scores/exp/mask while
  block i-1 runs its AV matmuls; AV matmuls are emitted first within
  each group so PE covers the eviction drain. The fexp group leads each
  block so the DVE work starts immediately.
"""

import math
import sys

sys.path.insert(0, "/opt/trn_rl_repo")

import numpy as np

B, N, D = 4, 4096, 128
R = N // 2            # rows per core
NK = N // 128         # 32 key tiles
NK_U = 13             # key tiles with device bit-unpack
NK_M = 26             # key tiles masked multiplicatively (0..NK_M-1)
NK_F = NK - NK_M      # fast-exp key tiles (NK_M..31)
RB = 512              # row granularity of the host-packed mask layout
NRB = R // RB
SCALE = 1.0 / math.sqrt(D)
EXP_BIAS = -8.0

# fast-exp: exp(z) ~ f16_bits(round(z*1024/ln2 + 15360 - ADJ))
LOG2_SC = (1 << 10) / math.log(2.0)     # 1477.3196
ALPHA = SCALE * LOG2_SC                 # psS = raw_dot * ALPHA
SQ_ALPHA = math.sqrt(ALPHA)             # host pre-scale per score operand
FEXP_ADJ = 50.0
FEXP_C = float(round((15 << 10) + EXP_BIAS * LOG2_SC - FEXP_ADJ))
ACT_SCALE = SCALE / ALPHA               # makes ACT see raw_dot*SCALE

# row blocks (offset, size): last two halved to shorten the AV drain tail
BLOCKS = [(0, 512), (512, 512), (1024, 512), (1536, 256), (1792, 256)]

# mask-mul spans (k0, nk); evict groups never cross span boundaries
MUL_SPANS = [(0, 9), (9, 9), (18, 8)]

# processing order: fexp tiles first
K_ORDER = list(range(NK_M, NK)) + list(range(NK_M))

_CACHE = {}


def _chunk_groups(max_w):
    """Chunk K_ORDER into evict groups of width <= max_w that never
    cross a mask-span boundary (fexp tiles form their own groups)."""
    bounds = {k0 for k0, _ in MUL_SPANS} | {k0 + nk for k0, nk in MUL_SPANS}
    groups = []
    cur = []
    for k in K_ORDER:
        if cur:
            same_kind = (cur[0] >= NK_M) == (k >= NK_M)
            crosses = k in bounds
            if (not same_kind) or crosses or len(cur) >= max_w or k != cur[-1] + 1:
                groups.append(cur)
                cur = []
        cur.append(k)
    groups.append(cur)
    return groups


def _build_nc(cfg):
    from concourse import bacc
    import concourse.mybir as mybir
    from concourse.tile import TileContext

    dt = mybir.dt

    nc = bacc.Bacc(None, target_bir_lowering=False)

    xt_d = nc.dram_tensor("xt", [D, N], dt.float16, kind="ExternalInput")
    xtr_d = nc.dram_tensor("xtr", [D, R], dt.float16, kind="ExternalInput")
    xaug_d = nc.dram_tensor("xaug", [N, D + 1], dt.bfloat16, kind="ExternalInput")
    # bit-packed 0/1 mask for key tiles 0..NK_U-1
    w_d = nc.dram_tensor("words", [NRB, 128, NK_U, 64], dt.uint16,
                         kind="ExternalInput")
    # fully-formed {0, 2^bit(row)} mask for key tiles NK_U..NK_M-1
    m2_d = nc.dram_tensor("m2", [128, NK_M - NK_U, R], dt.uint16,
                          kind="ExternalInput")
    # additive fast-exp mask for key tiles NK_M..31
    madd_d = nc.dram_tensor("madd", [128, NK_F, R], dt.int16,
                            kind="ExternalInput")
    o_d = nc.dram_tensor("o", [R, D], dt.float32, kind="ExternalOutput")

    with TileContext(nc) as tc:
        with (
            tc.tile_pool(name="singles", bufs=1) as singles,
            tc.tile_pool(name="words", bufs=2) as w_pool,
            tc.tile_pool(name="madd", bufs=2) as madd_pool,
            tc.tile_pool(name="mu", bufs=2) as mu_pool,
            tc.tile_pool(name="ptm", bufs=2) as ptm_pool,
            tc.tile_pool(name="pe", bufs=2) as pe_pool,
            tc.tile_pool(name="pef", bufs=2) as pef_pool,
            tc.tile_pool(name="outs", bufs=4) as out_pool,
            tc.tile_pool(name="small", bufs=4) as small_pool,
            tc.tile_pool(name="psS", bufs=2, space="PSUM") as psS_pool,
            tc.tile_pool(name="psO", bufs=2, space="PSUM") as psO_pool,
        ):
            ebias = singles.tile([128, 1], mybir.dt.float32)
            nc.vector.memset(ebias[:], EXP_BIAS)
            # warm the exp table while the init DMAs stream in
            warm = small_pool.tile([128, 1], mybir.dt.float32, tag="warm")
            nc.vector.memset(warm[:], 0.0)
            warm2 = small_pool.tile([128, 1], mybir.dt.float32, tag="warm")
            nc.scalar.activation(
                warm2[:], warm[:], mybir.ActivationFunctionType.Exp, scale=1.0
            )

            # init DMAs staggered by first consumption: the first evict
            # group is fexp over keys NK_M*128.., so its xt slice and
            # madd chunk lead their rings.
            xt_sb = singles.tile([D, N], dt.float16)
            xtr_sb = singles.tile([D, R], dt.float16)
            nc.sync.dma_start(out=xtr_sb[:, 0:512], in_=xtr_d[:, 0:512])
            nc.sync.dma_start(out=xt_sb[:, NK_M * 128:4096],
                              in_=xt_d[:, NK_M * 128:4096])
            madd_tiles = {}
            madd_tiles[0] = madd_pool.tile([128, NK_F, 512], dt.int16,
                                           tag="madd", name="madd_0")
            nc.gpsimd.dma_start(out=madd_tiles[0][:], in_=madd_d[:, :, 0:512])
            nc.sync.dma_start(out=xt_sb[:, 0:1536], in_=xt_d[:, 0:1536])
            w_tiles = {}
            w_tiles[0] = w_pool.tile([128, NK_U, 64], dt.uint16, tag="w",
                                     name="w_0")
            nc.gpsimd.dma_start(out=w_tiles[0][:], in_=w_d[0])
            nc.sync.dma_start(out=xtr_sb[:, 512:1024], in_=xtr_d[:, 512:1024])
            nc.sync.dma_start(out=xt_sb[:, 1536:NK_M * 128],
                              in_=xt_d[:, 1536:NK_M * 128])
            xaug_sb = singles.tile([128, NK, D + 1], dt.bfloat16)
            nc.gpsimd.dma_start(
                out=xaug_sb[:],
                in_=xaug_d[:, :].rearrange("(t p) d -> p t d", p=128),
            )
            nc.gpsimd.dma_start(out=xtr_sb[:, 1024:2048], in_=xtr_d[:, 1024:2048])

            NB = len(BLOCKS)
            ptm_prev = None
            pef_prev = None
            bs_prev = None
            off_prev = None

            K_FIRST, K_LAST = K_ORDER[0], K_ORDER[-1]

            def emit_av(psO, k, rc):
                if k >= NK_M:
                    lhsT = pef_prev[:, k - NK_M, rc * 128:(rc + 1) * 128].bitcast(
                        dt.float16
                    )
                else:
                    lhsT = ptm_prev[:, k, rc * 128:(rc + 1) * 128]
                nc.tensor.matmul(
                    psO[rc // 2][:, rc % 2, :],
                    lhsT=lhsT,
                    rhs=xaug_sb[:, k, :],
                    start=(k == K_FIRST),
                    stop=(k == K_LAST),
                )

            span_of = {}
            for k0, nkk in MUL_SPANS:
                for k in range(k0, k0 + nkk):
                    span_of[k] = k0

            for phase in range(NB + 1):
                ptm_cur = None
                pef_cur = None
                psO = None
                if phase < NB:
                    off, bs = BLOCKS[phase]
                    rb = off // RB
                    bit0 = (off % RB) // 64
                    nbits = bs // 64
                    m_u = mu_pool.tile([128, NK_M, bs], dt.uint16, tag="mu",
                                       name=f"mu_{phase}")
                    # host-formed mask slab for tiles NK_U..NK_M-1
                    nc.gpsimd.dma_start(
                        out=m_u[:, NK_U:NK_M, :],
                        in_=m2_d[:, :, off:off + bs],
                    )
                    # prefetch next block's madd / words
                    if phase + 1 < NB:
                        off_n, bs_n = BLOCKS[phase + 1]
                        madd_tiles[phase + 1] = madd_pool.tile(
                            [128, NK_F, bs_n], dt.int16, tag="madd",
                            name=f"madd_{phase + 1}"
                        )
                        nc.gpsimd.dma_start(
                            out=madd_tiles[phase + 1][:],
                            in_=madd_d[:, :, off_n:off_n + bs_n],
                        )
                        rb_n = off_n // RB
                        if rb_n != rb and rb_n not in w_tiles:
                            w_tiles[rb_n] = w_pool.tile(
                                [128, NK_U, 64], dt.uint16, tag="w",
                                name=f"w_{rb_n}"
                            )
                            nc.gpsimd.dma_start(out=w_tiles[rb_n][:],
                                                in_=w_d[rb_n])
                    # unpack this block's mask bits (key tiles 0..NK_U-1)
                    wt = w_tiles[rb]
                    for i in range(nbits):
                        nc.vector.tensor_scalar(
                            m_u[:, 0:NK_U, i * 64:(i + 1) * 64],
                            wt[:],
                            1 << (bit0 + i),
                            None,
                            mybir.AluOpType.bitwise_and,
                        )
                    ptm_cur = ptm_pool.tile([128, NK_M, bs], dt.bfloat16,
                                            tag="ptm", name=f"ptm_{phase}")
                    pef_cur = pef_pool.tile([128, NK_F, bs], dt.uint16,
                                            tag="pef", name=f"pef_{phase}")
                    madd_t = madd_tiles.get(phase)
                if phase >= 1:
                    # pairs of [128, 129] accumulators packed per PSUM bank
                    psO = [
                        psO_pool.tile(
                            [128, 2, D + 1], mybir.dt.float32,
                            tag="psO", name=f"psO_{phase}_{g}",
                        )
                        for g in range(bs_prev // 256)
                    ]

                if phase == NB:
                    # drain: rc-major AV bursts so each psO finishes early
                    for rc in range(bs_prev // 128):
                        for k in K_ORDER:
                            emit_av(psO, k, rc)
                        recip = small_pool.tile([128, 1], mybir.dt.float32,
                                                tag="recip", name=f"recipd_{rc}")
                        nc.vector.reciprocal(recip[:],
                                             psO[rc // 2][:, rc % 2, D:D + 1])
                        o_sb = out_pool.tile([128, D], mybir.dt.float32, tag="o",
                                             name=f"od_{rc}")
                        nc.scalar.activation(
                            o_sb[:], psO[rc // 2][:, rc % 2, 0:D],
                            mybir.ActivationFunctionType.Copy, scale=recip[:],
                        )
                        r0 = off_prev + rc * 128
                        nc.sync.dma_start(out=o_d[r0:r0 + 128, :], in_=o_sb[:])
                    break

                halves = 2 if bs == 512 else 1
                hs = bs // halves
                groups = _chunk_groups(3 if bs == 512 else 6)
                pe_span = {
                    k0: pe_pool.tile([128, nkk, bs], dt.bfloat16,
                                     tag=f"pe{k0}", name=f"pe_{phase}_{k0}")
                    for k0, nkk in MUL_SPANS
                }
                done = set()
                muls_emitted = set()
                # AV ops rc-major: the two accumulation streams sharing a
                # PSUM bank run sequentially (start..stop of rc even fully
                # precedes rc odd), never interleaved within a bank
                av_ops = []
                if phase >= 1:
                    for rc in range(bs_prev // 128):
                        for k in K_ORDER:
                            av_ops.append((rc, k))
                ng = len(groups)
                av_pos = 0
                for gi, gks in enumerate(groups):
                    # AV matmuls for the previous block first: PE has work
                    # while the eviction drains this group's scores
                    n_av = (len(av_ops) * (gi + 1)) // ng - (len(av_ops) * gi) // ng
                    for _ in range(n_av):
                        rc, k = av_ops[av_pos]
                        av_pos += 1
                        emit_av(psO, k, rc)
                    gw = len(gks)
                    ps = psS_pool.tile([128, gw, bs], mybir.dt.float32,
                                       tag="psS", name=f"psS_{phase}_{gi}")
                    for j, k in enumerate(gks):
                        nc.tensor.matmul(
                            ps[:, j, :],
                            lhsT=xt_sb[:, k * 128:(k + 1) * 128],
                            rhs=xtr_sb[:, off:off + bs],
                            start=True,
                            stop=True,
                        )
                    k0g = gks[0]
                    if k0g >= NK_M:
                        f0 = k0g - NK_M
                        nc.vector.tensor_tensor(
                            pef_cur[:, f0:f0 + gw, :],
                            ps[:, :, :],
                            madd_t[:, f0:f0 + gw, :],
                            mybir.AluOpType.add,
                        )
                    else:
                        k0s = span_of[k0g]
                        i0 = k0g - k0s
                        nc.scalar.activation(
                            pe_span[k0s][:, i0:i0 + gw, :],
                            ps[:, :, :],
                            mybir.ActivationFunctionType.Exp,
                            bias=ebias[:],
                            scale=ACT_SCALE,
                        )
                    done.update(gks)
                    # masked probs for spans whose evictions completed
                    for k0s, nkk in MUL_SPANS:
                        if k0s in muls_emitted:
                            continue
                        if all((k0s + t) in done for t in range(nkk)):
                            muls_emitted.add(k0s)
                            for h in range(halves):
                                nc.vector.tensor_tensor(
                                    ptm_cur[:, k0s:k0s + nkk,
                                            h * hs:(h + 1) * hs],
                                    pe_span[k0s][:, :, h * hs:(h + 1) * hs],
                                    m_u[:, k0s:k0s + nkk,
                                        h * hs:(h + 1) * hs],
                                    mybir.AluOpType.mult,
                                )
                if phase >= 1:
                    for rc in range(bs_prev // 128):
                        recip = small_pool.tile([128, 1], mybir.dt.float32,
                                                tag="recip",
                                                name=f"recip_{phase}_{rc}")
                        nc.vector.reciprocal(recip[:],
                                             psO[rc // 2][:, rc % 2, D:D + 1])
                        o_sb = out_pool.tile([128, D], mybir.dt.float32, tag="o",
                                             name=f"o_{phase}_{rc}")
                        nc.scalar.activation(
                            o_sb[:], psO[rc // 2][:, rc % 2, 0:D],
                            mybir.ActivationFunctionType.Copy, scale=recip[:],
                        )
                        r0 = off_prev + rc * 128
                        nc.sync.dma_start(out=o_d[r0:r0 + 128, :], in_=o_sb[:])
                ptm_prev = ptm_cur
                pef_prev = pef_cur
                bs_prev = bs
                off_prev = off
    nc.finalize()
    return nc


def _get_nc():
    if "nc" not in _CACHE:
        _CACHE["nc"] = _build_nc(None)
    return _CACHE["nc"]


def make_in_maps(input, adj):
    """Host-side shard/layout prep: one input map per core."""
    import ml_dtypes

    input = np.asarray(input, dtype=np.float32)
    adj = np.asarray(adj)

    in_maps = []
    for core in range(8):
        b, h = core // 2, core % 2
        xb = input[b]                                    # [N, D]
        xs = (xb.T * SQ_ALPHA).astype(np.float16)        # pre-scaled scores
        xt = np.ascontiguousarray(xs)
        xtr = np.ascontiguousarray(xs[:, h * R:(h + 1) * R])
        xaug = np.concatenate([xb, np.ones((N, 1), np.float32)], axis=1)
        xaug = np.ascontiguousarray(xaug).astype(ml_dtypes.bfloat16)
        s = adj[b][h * R:(h + 1) * R, :] > 0             # [R rows, N cols]
        rows = np.arange(R)
        brow = ((rows % RB) // 64).astype(np.float64)    # bit index per row

        # words[rb, p, k, j]: bit i = mask row rb*512+i*64+j, key k*128+p
        sb = s[:, : NK_U * 128].reshape(NRB, 8, 64, NK_U, 128)
        words = (
            sb.astype(np.uint16)
            << np.arange(8, dtype=np.uint16)[None, :, None, None, None]
        ).sum(axis=1, dtype=np.uint16)                   # [rb, j, k, p]
        words = np.ascontiguousarray(words.transpose(0, 3, 2, 1))  # [rb,p,k,j]

        # m2[p, kt, r]: fully-formed {0, 2^bit(row)} for tiles NK_U..NK_M-1
        sm = s[:, NK_U * 128: NK_M * 128]                # [R, 13*128]
        m2 = np.where(sm, (2.0 ** brow)[:, None], 0.0)   # [R, cols]
        m2 = m2.reshape(R, NK_M - NK_U, 128).transpose(2, 1, 0)
        m2 = np.ascontiguousarray(m2).astype(np.uint16)

        # madd[p, kt, r] for keys NK_M*128.. : additive fast-exp mask
        base = FEXP_C + 1024.0 * brow                    # [R]
        m3 = s[:, NK_M * 128:]                           # [R, NK_F*128]
        madd = np.where(m3, base[:, None], -32768.0)     # [R, cols]
        if h == 1:
            # diagonal keys (global row == key) in the fexp range; clamp
            # so psS_diag + madd stays below the fp16 inf bit region
            xs64 = xs.astype(np.float64)
            g = np.arange(NK_M * 128, 4096)              # global fexp keys
            r_idx = g - R                                # local row
            ps_diag = (xs64[:, g] * xs64[:, g]).sum(axis=0)
            cap = 31500.0 - ps_diag
            col = g - NK_M * 128
            cur = madd[r_idx, col]
            madd[r_idx, col] = np.where(
                m3[r_idx, col], np.minimum(cur, cap), cur
            )
        madd = madd.reshape(R, NK_F, 128).transpose(2, 1, 0)  # [p, kt, r]
        madd = np.ascontiguousarray(np.round(madd)).astype(np.int16)
        in_maps.append({
            "xt": xt, "xtr": xtr, "xaug": xaug,
            "words": words, "m2": m2, "madd": madd,
        })
    return in_maps


def run_device(in_maps, trace=False, trace_cores=None):
    import concourse.bass_utils as bass_utils

    if trace:
        bass_utils.upload_artifacts = lambda tmpdir: ""  # no bucket in sandbox
    nc = _get_nc()
    return bass_utils.run_bass_kernel_spmd(
        nc, in_maps, list(range(8)), trace=trace, trace_cores=trace_cores
    )


def kernel(input, adj):
    res = run_device(make_in_maps(input, adj))
    out = np.empty((B, N, D), dtype=np.float32)
    for core in range(8):
        b, h = core // 2, core % 2
        out[b, h * R:(h + 1) * R, :] = res.results[core]["o"]
    return out
